# revision 1
# baseline (speedup 1.0000x reference)
"""Trainium2 Bass kernel for nn_CHILDREN_TENSOR (gnn_message_passing).

Problem: nodes [16, 2048, 128] f32, children [16, 2048, 32] int32.
Output [16, 2048, 32, 128] f32: out[b, n, c, :] = lookup[b, children[b,n,c], :]
where lookup = nodes with row 0 zeroed per batch.

Strategy: data-parallel over the batch dim on 8 NeuronCores (2 batch
elements per core). Per core, a pipeline of SWDGE dma_gather calls
(HBM node table -> SBUF, 512 B per row, 1024 rows per call so the 64
descriptors per SDMA engine fit one packet) fills 8192-row SBUF buffers
that HWDGE stores flush to HBM as 4 MB contiguous-per-partition writes.
Host-side index preprocessing permutes children so each SBUF partition
accumulates rows contiguous in the output. Gathers alternate between two
SWDGE queues so descriptor generation overlaps draining; 4 SBUF buffers
let gathers and stores overlap.
"""

import sys

for _p in ("/opt/trn_rl_repo",):
    if _p not in sys.path:
        sys.path.insert(0, _p)

from contextlib import ExitStack

import numpy as np

import concourse.bacc as bacc
import concourse.mybir as mybir
from concourse.bass_utils import run_bass_kernel_spmd

# Problem constants (hardcoded per harness contract).
B, N, C, F = 16, 2048, 32, 128
N_CORES = 8
B_PER_CORE = B // N_CORES            # 2
ROWS_PER_BATCH = N * C               # 65536 gathered rows per batch element
ROWS_PER_CORE = B_PER_CORE * ROWS_PER_BATCH          # 131072

GATHER_ROWS = 1024                   # rows per dma_gather (64 descs/engine)
G_SUB = GATHER_ROWS // 128           # 8 free-dim blocks per gather
IDX_COLS = GATHER_ROWS // 16         # 64 idx columns per gather

GROUP_ROWS = 8192                    # rows per store
G = GROUP_ROWS // 128                # 64 free-dim blocks per group buffer
GATHERS_PER_GROUP = GROUP_ROWS // GATHER_ROWS        # 8
N_GROUPS = ROWS_PER_CORE // GROUP_ROWS               # 16 per iteration
N_GATHERS = N_GROUPS * GATHERS_PER_GROUP             # 128 per iteration

NSEMS = 16                           # rotating sem pool depth
NBUFS = 4                            # group-sized SBUF buffers


def build_nc(repeat=1, timing_build=False, mode="full"):
    nc = bacc.Bacc(
        "TRN2", debug=False, target_bir_lowering=False,
        num_swdge_queues=2,
    )

    nodes = nc.dram_tensor(
        "nodes", [B_PER_CORE, N, F], mybir.dt.float32,
        kind="Internal" if timing_build else "ExternalInput",
    )
    idxs = nc.dram_tensor(
        "idxs", [128, N_GATHERS * IDX_COLS], mybir.dt.int16, kind="ExternalInput"
    )
    out = nc.dram_tensor(
        "out", [ROWS_PER_CORE, F], mybir.dt.float32,
        kind="Internal" if timing_build else "ExternalOutput",
    )
    tok = (
        nc.dram_tensor("tok", [1, F], mybir.dt.float32, kind="ExternalOutput")
        if timing_build else None
    )

    do_gather = mode in ("full", "gather")
    do_store = mode in ("full", "store")

    with (
        nc.sbuf_tensor(
            "idx_sb", [128, N_GATHERS * IDX_COLS], mybir.dt.int16
        ) as idx_sb,
        nc.sbuf_tensor("buf", [128, NBUFS, G, F], mybir.dt.float32) as buf,
        nc.semaphore("load_sem") as load_sem,
        ExitStack() as stack,
        nc.Block() as block,
    ):
        gather_sems = [
            stack.enter_context(nc.semaphore(f"gather_sem{i}"))
            for i in range(NSEMS)
        ]
        store_sems = [
            stack.enter_context(nc.semaphore(f"store_sem{i}"))
            for i in range(NSEMS)
        ]
        total_groups = N_GROUPS * repeat

        # Per-group gather-sem target: 8 gathers x 16 each.
        def g_target(gs):
            return 16 * GATHERS_PER_GROUP * (gs // NSEMS + 1)

        def s_target(gs):
            return 16 * (gs // NSEMS + 1)

        @block.gpsimd
        def _(gpsimd):
            gpsimd.dma_start(idx_sb[:], idxs[:]).then_inc(load_sem, 16)
            gpsimd.wait_ge(load_sem, 16)
            if do_gather:
                for gs in range(total_groups):
                    s = gs % N_GROUPS
                    b = s // (N_GROUPS // B_PER_CORE)
                    if gs >= NBUFS:
                        dep = store_sems if do_store else gather_sems
                        tgt = (s_target if do_store else g_target)(gs - NBUFS)
                        gpsimd.wait_ge(dep[(gs - NBUFS) % NSEMS], tgt)
                    for j in range(GATHERS_PER_GROUP):
                        gi = s * GATHERS_PER_GROUP + j
                        col = gi * IDX_COLS
                        gpsimd.dma_gather(
                            buf[:, gs % NBUFS, j * G_SUB:(j + 1) * G_SUB],
                            nodes[b],
                            idx_sb[:, col:col + IDX_COLS],
                            GATHER_ROWS,
                            GATHER_ROWS,
                            F,
                            queue_num=gs % 2,
                        ).then_inc(gather_sems[gs % NSEMS], 16)

        @block.sync
        def _(sync):
            # Merged 2-D APs on both sides: per partition one contiguous
            # 32 KB run -> large descriptors.
            out_v = out.rearrange("(s p gf) f -> s p (gf f)", p=128, gf=G)
            buf_v = buf.rearrange("p n g f -> p n (g f)")
            if do_store:
                for gs in range(total_groups):
                    s = gs % N_GROUPS
                    if do_gather:
                        sync.wait_ge(gather_sems[gs % NSEMS], g_target(gs))
                    elif gs >= NBUFS:
                        sync.wait_ge(
                            store_sems[(gs - NBUFS) % NSEMS],
                            s_target(gs - NBUFS),
                        )
                    sync.dma_start(
                        out_v[s], buf_v[:, gs % NBUFS]
                    ).then_inc(store_sems[gs % NSEMS], 16)
                for i in range(NSEMS):
                    sync.wait_ge(store_sems[i], 16 * (total_groups // NSEMS))
            elif do_gather:
                for i in range(NSEMS):
                    sync.wait_ge(
                        gather_sems[i],
                        16 * GATHERS_PER_GROUP * (total_groups // NSEMS),
                    )
            if tok is not None:
                sync.dma_start(tok[:], buf[:1, 0, 0, :]).then_inc(load_sem, 16)
                sync.wait_ge(load_sem, 32)

    nc.compile()
    return nc


def make_in_maps(nodes, children):
    """Host-side shard + index preprocessing.

    Group buffer layout: partition p, block g (64 per group) holds output
    row group_base + p*64 + g. Gather j of a group fills blocks
    g = 8j..8j+7; within gather j, fed slot j_local = g_sub*128 + p lands
    at dst[p, g_sub], so idx_lin[g_sub*128 + p] must be
    children_flat[group_base + p*64 + 8j + g_sub]. dma_gather reads
    indices wrapped over 16 partitions (replicated to all 8 Q7 core
    groups): idx_sb[l, s] = idx_lin[s*16 + l].
    """
    nodes_z = np.ascontiguousarray(np.asarray(nodes), dtype=np.float32).copy()
    nodes_z[:, 0, :] = 0.0
    ch = np.ascontiguousarray(np.asarray(children)).astype(np.int16)

    in_maps = []
    for core in range(N_CORES):
        nb = nodes_z[core * B_PER_CORE:(core + 1) * B_PER_CORE]
        cb = ch[core * B_PER_CORE:(core + 1) * B_PER_CORE].reshape(
            ROWS_PER_CORE
        )
        # row s*8192 + p*64 + 8j + g_sub  ->  [s, p, j, g_sub]
        r = cb.reshape(N_GROUPS, 128, GATHERS_PER_GROUP, G_SUB)
        # gather (s, j) linear layout [g_sub*128 + p]  ->  [s, j, g_sub, p]
        r = r.transpose(0, 2, 3, 1).reshape(N_GATHERS, GATHER_ROWS)
        # wrap 16: idx_sb16[l, col] = idx_lin[col*16 + l]
        w = r.reshape(N_GATHERS, IDX_COLS, 16)
        w = w.transpose(2, 0, 1).reshape(16, N_GATHERS * IDX_COLS)
        idx_t = np.tile(w, (8, 1)).astype(np.int16)
        in_maps.append({"nodes": np.ascontiguousarray(nb), "idxs": idx_t})
    return in_maps


_NC_CACHE = None


def kernel(nodes, children, feature_size=None):
    global _NC_CACHE
    if _NC_CACHE is None:
        _NC_CACHE = build_nc()
    nc = _NC_CACHE

    in_maps = make_in_maps(nodes, children)
    res = run_bass_kernel_spmd(nc, in_maps, list(range(N_CORES))).results

    out = np.empty((B, N, C, F), np.float32)
    for core in range(N_CORES):
        out[core * B_PER_CORE:(core + 1) * B_PER_CORE] = (
            res[core]["out"].reshape(B_PER_CORE, N, C, F)
        )
    return out



# revision 2
# speedup vs baseline: 42.1192x; 42.1192x over previous
"""Trainium2 Bass kernel v2 for nn_CHILDREN_TENSOR (gnn_message_passing).

Problem: nodes [16, 2048, 128] f32, children [16, 2048, 32] int32.
Output [16, 2048, 32, 128] f32: out[b, n, c, :] = lookup[b, children[b,n,c], :]
where lookup = nodes with row 0 zeroed per batch.

Strategy (data-parallel over batch, 2 per core): keep the whole node
table in SBUF feature-major ([128 feat-partitions x 4096 rows]) and do
the gather ON-CHIP with gpsimd ap_gather (free-dim gather, identical
index stream for all partition groups). Gathered columns are transposed
back to row-major 128x128 tiles on the PE (is_transpose matmul against a
fp32 identity - a pure permutation, bit-exact), drained PSUM->SBUF by
ACT and DVE in 512-column blocks, and stored to HBM as 4 MB
contiguous-per-partition HWDGE writes. DMA then only carries the 2 MB
table + 2 MB indices in and the 64 MB result out - the 64 MB random
HBM gather read of the dma_gather design is gone.
"""

import sys

for _p in ("/opt/trn_rl_repo",):
    if _p not in sys.path:
        sys.path.insert(0, _p)

from contextlib import ExitStack

import numpy as np

import concourse.bacc as bacc
import concourse.mybir as mybir
from concourse.bass_utils import run_bass_kernel_spmd

# Problem constants (hardcoded per harness contract).
B, N, C, F = 16, 2048, 32, 128
N_CORES = 8
B_PER_CORE = B // N_CORES            # 2
ROWS_PER_CORE = B_PER_CORE * N * C   # 131072 output rows per core
TBL_COLS = B_PER_CORE * N            # 4096 table columns (feature-major)

CHUNK = 2048                         # gather columns per ap_gather call
NCHUNK = ROWS_PER_CORE // CHUNK      # 64 per iteration
TPC = CHUNK // 128                   # 16 transpose tiles per chunk
GROUP_ROWS = 8192                    # rows per store
GT = GROUP_ROWS // 128               # 64 tiles per store group
N_GROUPS = ROWS_PER_CORE // GROUP_ROWS               # 16 stores per iteration
BLOCKS = ROWS_PER_CORE // 512        # 256 copy blocks (4 tiles) per iteration
BPG = GT // 4                        # 16 copy blocks per store group

NSEMS = 16
NBUFS = 3                            # store-group SBUF buffers
NPSUM = 6                            # rotating PSUM banks of [128, 512]
GSLOTS = 6                           # gather-chunk pipeline depth


def build_nc(repeat=1, timing_build=False, mode="full",
             do_gather=True, do_pe=True, do_copy=True, do_store=True):
    if mode == "store":
        do_gather = do_pe = do_copy = False
    elif mode == "gather":
        do_pe = do_copy = do_store = False
    elif mode == "nostore":
        do_store = False
    elif mode == "compute":
        do_gather = do_store = False
    nc = bacc.Bacc("TRN2", debug=False, target_bir_lowering=False)

    table = nc.dram_tensor(
        "table", [128, TBL_COLS], mybir.dt.float32,
        kind="Internal" if timing_build else "ExternalInput",
    )
    idxs = nc.dram_tensor(
        "idxs", [128, ROWS_PER_CORE // 16], mybir.dt.int16, kind="ExternalInput"
    )
    ident = nc.dram_tensor("ident", [128, 128], mybir.dt.float32,
                           kind="ExternalInput")
    out = nc.dram_tensor(
        "out", [ROWS_PER_CORE, F], mybir.dt.float32,
        kind="Internal" if timing_build else "ExternalOutput",
    )
    tok = (
        nc.dram_tensor("tok", [1, F], mybir.dt.float32, kind="ExternalOutput")
        if timing_build else None
    )

    with (
        nc.sbuf_tensor("table_sb", [128, TBL_COLS], mybir.dt.float32) as table_sb,
        nc.sbuf_tensor(
            "idx_sb", [128, ROWS_PER_CORE // 16], mybir.dt.int16
        ) as idx_sb,
        nc.sbuf_tensor("ident_sb", [128, 128], mybir.dt.float32) as ident_sb,
        nc.sbuf_tensor("gbuf", [128, GSLOTS, CHUNK], mybir.dt.float32) as gbuf,
        nc.sbuf_tensor("buf", [128, NBUFS, GT, F], mybir.dt.float32) as buf,
        nc.semaphore("load_sem") as load_sem,
        ExitStack() as stack,
        nc.Block() as block,
    ):
        psum = [
            stack.enter_context(
                nc.psum_tensor(f"ps{i}", [128, 512], mybir.dt.float32)
            )
            for i in range(NPSUM)
        ]
        gather_sems = [
            stack.enter_context(nc.semaphore(f"gather_sem{i}"))
            for i in range(NSEMS)
        ]
        blkready_sems = [
            stack.enter_context(nc.semaphore(f"blkready_sem{i}"))
            for i in range(NSEMS)
        ]
        copydone_sems = [
            stack.enter_context(nc.semaphore(f"copydone_sem{i}"))
            for i in range(NSEMS)
        ]
        store_sems = [
            stack.enter_context(nc.semaphore(f"store_sem{i}"))
            for i in range(NSEMS)
        ]

        def rnd(i):
            return i // NSEMS + 1

        @block.sync
        def _(sync):
            sync.dma_start(idx_sb[:], idxs[:]).then_inc(load_sem, 16)
            sync.dma_start(table_sb[:], table[:]).then_inc(load_sem, 16)
            sync.dma_start(ident_sb[:], ident[:]).then_inc(load_sem, 16)
            # Merged 2-D APs on both sides: per partition one contiguous
            # 32 KB run -> large descriptors.
            out_v = out.rearrange("(s p gf) f -> s p (gf f)", p=128, gf=GT)
            buf_v = buf.rearrange("p n g f -> p n (g f)")
            if do_store:
                for gs in range(repeat * N_GROUPS):
                    s = gs % N_GROUPS
                    if do_copy:
                        for q in range(gs * BPG, (gs + 1) * BPG):
                            sync.wait_ge(copydone_sems[q % NSEMS], rnd(q))
                    sync.dma_start(
                        out_v[s], buf_v[:, gs % NBUFS]
                    ).then_inc(store_sems[gs % NSEMS], 16)
                for i in range(NSEMS):
                    sync.wait_ge(
                        store_sems[i], 16 * (repeat * N_GROUPS // NSEMS)
                    )
            elif do_copy:
                for i in range(NSEMS):
                    sync.wait_ge(
                        copydone_sems[i], repeat * BLOCKS // NSEMS
                    )
            elif do_gather:
                for i in range(NSEMS):
                    sync.wait_ge(
                        gather_sems[i], repeat * NCHUNK // NSEMS
                    )
            if tok is not None:
                sync.dma_start(tok[:], buf[:1, 0, 0, :]).then_inc(load_sem, 16)
                sync.wait_ge(load_sem, 64)

        @block.gpsimd
        def _(gpsimd):
            if not do_gather:
                return
            gpsimd.wait_ge(load_sem, 48)
            for gc in range(repeat * NCHUNK):
                c = gc % NCHUNK
                bpc = CHUNK // 512
                if gc >= GSLOTS and do_copy:
                    for q in range((gc - GSLOTS) * bpc, (gc - GSLOTS + 1) * bpc):
                        gpsimd.wait_ge(copydone_sems[q % NSEMS], rnd(q))
                # chunks never straddle the batch boundary; use the
                # per-batch 2048-column table slice with batch-local idxs
                b = c // (NCHUNK // B_PER_CORE)
                gpsimd.ap_gather(
                    gbuf[:, gc % GSLOTS],
                    table_sb[:, b * N:(b + 1) * N],
                    idx_sb[:, c * (CHUNK // 16):(c + 1) * (CHUNK // 16)],
                    128,          # channels
                    N,            # num_elems
                    1,            # d
                    CHUNK,        # num_idxs
                ).then_inc(gather_sems[gc % NSEMS], 1)

        @block.tensor
        def _(tensor):
            if not do_pe:
                return
            tensor.wait_ge(load_sem, 48)
            for gk in range(repeat * NCHUNK * TPC):
                gc, t = divmod(gk, TPC)
                q = gk // 4          # global 4-tile copy block
                if t == 0 and do_gather:
                    tensor.wait_ge(gather_sems[gc % NSEMS], rnd(gc))
                if gk % 4 == 0 and q >= NPSUM and do_copy:
                    tensor.wait_ge(copydone_sems[(q - NPSUM) % NSEMS],
                                   rnd(q - NPSUM))
                mm = tensor.matmul(
                    psum[q % NPSUM][:, (gk % 4) * 128:(gk % 4) * 128 + 128],
                    gbuf[:, gc % GSLOTS, t * 128:(t + 1) * 128],
                    ident_sb[:],
                    is_transpose=True,
                    start=True,
                    stop=True,
                )
                mm.then_inc(blkready_sems[q % NSEMS], 1)

        def copy_body(eng, parity):
            if not do_copy:
                return
            for q in range(repeat * BLOCKS):
                if q % 2 != parity:
                    continue
                gq = q // BPG        # global store group
                if do_pe:
                    eng.wait_ge(blkready_sems[q % NSEMS], 4 * rnd(q))
                if gq >= NBUFS and do_store:
                    eng.wait_ge(store_sems[(gq - NBUFS) % NSEMS],
                                16 * rnd(gq - NBUFS))
                qq = q % BPG
                dst = buf[:, gq % NBUFS, qq * 4:(qq + 1) * 4]
                src = psum[q % NPSUM][:]
                cp = (eng.copy(dst, src) if parity == 0
                      else eng.tensor_copy(dst, src))
                cp.then_inc(copydone_sems[q % NSEMS], 1)

        @block.scalar
        def _(scalar):
            copy_body(scalar, 0)

        @block.vector
        def _(vector):
            copy_body(vector, 1)

    nc.compile()
    return nc


def make_in_maps(nodes, children):
    """Host-side shard + layout preprocessing.

    Gather stream position c = s*8192 + t*128 + p produces output row
    s*8192 + p*64 + t (so the PE tile transpose + contiguous store land
    rows in natural order). ap_gather unwraps indices per 16-partition
    group as idx[16g + k%16, base + k//16], identical for all 8 groups.
    """
    nodes_z = np.ascontiguousarray(np.asarray(nodes), dtype=np.float32).copy()
    nodes_z[:, 0, :] = 0.0
    ch = np.asarray(children).astype(np.int64)
    ident = np.ascontiguousarray(np.eye(128, dtype=np.float32))

    in_maps = []
    for core in range(N_CORES):
        nb = nodes_z[core * B_PER_CORE:(core + 1) * B_PER_CORE]
        # feature-major table [128, 4096]
        table = np.ascontiguousarray(
            nb.transpose(2, 0, 1).reshape(F, TBL_COLS).astype(np.float32)
        )
        cb = ch[core * B_PER_CORE:(core + 1) * B_PER_CORE]
        # batch-local indices: each ap_gather call uses its batch's
        # 2048-column table slice
        flat = cb.reshape(ROWS_PER_CORE)
        # row r = s*8192 + p*64 + t  ->  stream position s*8192 + t*128 + p
        a = flat.reshape(N_GROUPS, 128, GT)          # [s, p, t]
        stream = a.transpose(0, 2, 1).reshape(ROWS_PER_CORE)  # [s, t, p]
        # wrap-16: idx16[l, j] = stream[j*16 + l]
        w = stream.reshape(ROWS_PER_CORE // 16, 16).T        # [16, 8192]
        idx16 = np.tile(w, (8, 1)).astype(np.int16)
        in_maps.append(
            {"table": table, "idxs": np.ascontiguousarray(idx16),
             "ident": ident}
        )
    return in_maps


_NC_CACHE = None


def kernel(nodes, children, feature_size=None):
    global _NC_CACHE
    if _NC_CACHE is None:
        _NC_CACHE = build_nc()
    nc = _NC_CACHE

    in_maps = make_in_maps(nodes, children)
    res = run_bass_kernel_spmd(nc, in_maps, list(range(N_CORES))).results

    out = np.empty((B, N, C, F), np.float32)
    for core in range(N_CORES):
        out[core * B_PER_CORE:(core + 1) * B_PER_CORE] = (
            res[core]["out"].reshape(B_PER_CORE, N, C, F)
        )
    return out


# revision 3
# speedup vs baseline: 43.9386x; 1.0432x over previous
"""Trainium2 Bass kernel v2 for nn_CHILDREN_TENSOR (gnn_message_passing).

Problem: nodes [16, 2048, 128] f32, children [16, 2048, 32] int32.
Output [16, 2048, 32, 128] f32: out[b, n, c, :] = lookup[b, children[b,n,c], :]
where lookup = nodes with row 0 zeroed per batch.

Strategy (data-parallel over batch, 2 per core): keep the whole node
table in SBUF feature-major ([128 feat-partitions x 4096 rows]) and do
the gather ON-CHIP with gpsimd ap_gather (free-dim gather, identical
index stream for all partition groups). Gathered columns are transposed
back to row-major 128x128 tiles on the PE (is_transpose matmul against a
fp32 identity - a pure permutation, bit-exact), drained PSUM->SBUF by
ACT and DVE in 512-column blocks, and stored to HBM as 4 MB
contiguous-per-partition HWDGE writes. DMA then only carries the 2 MB
table + 2 MB indices in and the 64 MB result out - the 64 MB random
HBM gather read of the dma_gather design is gone.
"""

import sys

for _p in ("/opt/trn_rl_repo",):
    if _p not in sys.path:
        sys.path.insert(0, _p)

from contextlib import ExitStack

import numpy as np

import concourse.bacc as bacc
import concourse.mybir as mybir
from concourse.bass_utils import run_bass_kernel_spmd

# Problem constants (hardcoded per harness contract).
B, N, C, F = 16, 2048, 32, 128
N_CORES = 8
B_PER_CORE = B // N_CORES            # 2
ROWS_PER_CORE = B_PER_CORE * N * C   # 131072 output rows per core
TBL_COLS = B_PER_CORE * N            # 4096 table columns (feature-major)

CHUNK = 2048                         # gather columns per ap_gather call
NCHUNK = ROWS_PER_CORE // CHUNK      # 64 per iteration
TPC = CHUNK // 128                   # 16 transpose tiles per chunk
GROUP_ROWS = 8192                    # rows per store
GT = GROUP_ROWS // 128               # 64 tiles per store group
N_GROUPS = ROWS_PER_CORE // GROUP_ROWS               # 16 stores per iteration
BLOCKS = ROWS_PER_CORE // 512        # 256 copy blocks (4 tiles) per iteration
BPG = GT // 4                        # 16 copy blocks per store group

NSEMS = 16
NBUFS = 3                            # store-group SBUF buffers
NPSUM = 6                            # rotating PSUM banks of [128, 512]
GSLOTS = 6                           # gather-chunk pipeline depth


def build_nc(repeat=1, timing_build=False, mode="full",
             do_gather=True, do_pe=True, do_copy=True, do_store=True):
    if mode == "store":
        do_gather = do_pe = do_copy = False
    elif mode == "gather":
        do_pe = do_copy = do_store = False
    elif mode == "nostore":
        do_store = False
    elif mode == "compute":
        do_gather = do_store = False
    nc = bacc.Bacc("TRN2", debug=False, target_bir_lowering=False)

    table = nc.dram_tensor(
        "table", [128, TBL_COLS], mybir.dt.float32,
        kind="Internal" if timing_build else "ExternalInput",
    )
    idxs = nc.dram_tensor(
        "idxs", [128, ROWS_PER_CORE // 16], mybir.dt.int16, kind="ExternalInput"
    )
    ident = nc.dram_tensor("ident", [128, 128], mybir.dt.float32,
                           kind="ExternalInput")
    out = nc.dram_tensor(
        "out", [ROWS_PER_CORE, F], mybir.dt.float32,
        kind="Internal" if timing_build else "ExternalOutput",
    )
    tok = (
        nc.dram_tensor("tok", [1, F], mybir.dt.float32, kind="ExternalOutput")
        if timing_build else None
    )

    with (
        nc.sbuf_tensor("table_sb", [128, TBL_COLS], mybir.dt.float32) as table_sb,
        nc.sbuf_tensor(
            "idx_sb", [128, ROWS_PER_CORE // 16], mybir.dt.int16
        ) as idx_sb,
        nc.sbuf_tensor("ident_sb", [128, 128], mybir.dt.float32) as ident_sb,
        nc.sbuf_tensor("gbuf", [128, GSLOTS, CHUNK], mybir.dt.float32) as gbuf,
        nc.sbuf_tensor("buf", [128, NBUFS, GT, F], mybir.dt.float32) as buf,
        nc.semaphore("load_sem") as load_sem,
        nc.semaphore("qstore_sem") as qstore_sem,
        nc.semaphore("tb0_sem") as tb0_sem,
        nc.semaphore("tb1_sem") as tb1_sem,
        nc.semaphore("idxp0") as idxp0,
        nc.semaphore("idxp1") as idxp1,
        nc.semaphore("idxp2") as idxp2,
        nc.semaphore("idxp3") as idxp3,
        ExitStack() as stack,
        nc.Block() as block,
    ):
        psum = [
            stack.enter_context(
                nc.psum_tensor(f"ps{i}", [128, 512], mybir.dt.float32)
            )
            for i in range(NPSUM)
        ]
        gather_sems = [
            stack.enter_context(nc.semaphore(f"gather_sem{i}"))
            for i in range(NSEMS)
        ]
        blkready_sems = [
            stack.enter_context(nc.semaphore(f"blkready_sem{i}"))
            for i in range(NSEMS)
        ]
        copydone_sems = [
            stack.enter_context(nc.semaphore(f"copydone_sem{i}"))
            for i in range(NSEMS)
        ]
        store_sems = [
            stack.enter_context(nc.semaphore(f"store_sem{i}"))
            for i in range(NSEMS)
        ]

        def rnd(i):
            return i // NSEMS + 1

        IDX_PIECES = 4
        IP_COLS = (ROWS_PER_CORE // 16) // IDX_PIECES
        CHUNKS_PER_PIECE = NCHUNK // IDX_PIECES

        @block.sync
        def _(sync):
            # batch-0 table + ident + idx piece 0 first so the gather and
            # PE pipelines start as early as possible; the rest streams in
            # behind them on dedicated semaphores.
            sync.dma_start(table_sb[:, :N], table[:, :N]).then_inc(tb0_sem, 16)
            sync.dma_start(ident_sb[:], ident[:]).then_inc(load_sem, 16)
            idxp_sems = [idxp0, idxp1, idxp2, idxp3]
            sync.dma_start(idx_sb[:, :IP_COLS],
                           idxs[:, :IP_COLS]).then_inc(idxp0, 16)
            sync.dma_start(table_sb[:, N:], table[:, N:]).then_inc(tb1_sem, 16)
            for ip in range(1, IDX_PIECES):
                sync.dma_start(
                    idx_sb[:, ip * IP_COLS:(ip + 1) * IP_COLS],
                    idxs[:, ip * IP_COLS:(ip + 1) * IP_COLS],
                ).then_inc(idxp_sems[ip], 16)
            # Merged 2-D APs on both sides: per partition one contiguous
            # 32 KB run -> large descriptors.
            out_v = out.rearrange("(s p gf) f -> s p (gf f)", p=128, gf=GT)
            buf_v = buf.rearrange("p n g f -> p n (g f)")
            if do_store:
                for gs in range(repeat * N_GROUPS):
                    s = gs % N_GROUPS
                    if (gs < 2 or gs == repeat * N_GROUPS - 1) and do_copy:
                        # pipeline fill: stream the first groups out in
                        # quarter stores so DMA starts right behind the
                        # first copies. HWDGE is FIFO per engine, so only
                        # the last quarter needs the semaphore.
                        qsz = (GT // 4) * F
                        for k in range(4):
                            for q in range(gs * BPG + 4 * k,
                                           gs * BPG + 4 * k + 4):
                                sync.wait_ge(copydone_sems[q % NSEMS],
                                             rnd(q))
                            st = sync.dma_start(
                                out_v[s][:, k * qsz:(k + 1) * qsz],
                                buf_v[:, gs % NBUFS, k * qsz:(k + 1) * qsz],
                            )
                            if k == 3:
                                st.then_inc(store_sems[gs % NSEMS], 16)
                            else:
                                st.then_inc(qstore_sem, 16)
                        continue
                    if do_copy:
                        for q in range(gs * BPG, (gs + 1) * BPG):
                            sync.wait_ge(copydone_sems[q % NSEMS], rnd(q))
                    sync.dma_start(
                        out_v[s], buf_v[:, gs % NBUFS]
                    ).then_inc(store_sems[gs % NSEMS], 16)
                for i in range(NSEMS):
                    sync.wait_ge(
                        store_sems[i], 16 * (repeat * N_GROUPS // NSEMS)
                    )
            elif do_copy:
                for i in range(NSEMS):
                    sync.wait_ge(
                        copydone_sems[i], repeat * BLOCKS // NSEMS
                    )
            elif do_gather:
                for i in range(NSEMS):
                    sync.wait_ge(
                        gather_sems[i], repeat * NCHUNK // NSEMS
                    )
            if tok is not None:
                sync.dma_start(tok[:], buf[:1, 0, 0, :]).then_inc(load_sem, 16)
                sync.wait_ge(load_sem, 32)

        @block.gpsimd
        def _(gpsimd):
            if not do_gather:
                return
            idxp_sems = [idxp0, idxp1, idxp2, idxp3]
            gpsimd.wait_ge(tb0_sem, 16)
            for gc in range(repeat * NCHUNK):
                c = gc % NCHUNK
                if gc < NCHUNK and c % CHUNKS_PER_PIECE == 0:
                    gpsimd.wait_ge(idxp_sems[c // CHUNKS_PER_PIECE], 16)
                if gc == NCHUNK // B_PER_CORE:
                    gpsimd.wait_ge(tb1_sem, 16)
                bpc = CHUNK // 512
                if gc >= GSLOTS and do_copy:
                    for q in range((gc - GSLOTS) * bpc, (gc - GSLOTS + 1) * bpc):
                        gpsimd.wait_ge(copydone_sems[q % NSEMS], rnd(q))
                # chunks never straddle the batch boundary; use the
                # per-batch 2048-column table slice with batch-local idxs
                b = c // (NCHUNK // B_PER_CORE)
                gpsimd.ap_gather(
                    gbuf[:, gc % GSLOTS],
                    table_sb[:, b * N:(b + 1) * N],
                    idx_sb[:, c * (CHUNK // 16):(c + 1) * (CHUNK // 16)],
                    128,          # channels
                    N,            # num_elems
                    1,            # d
                    CHUNK,        # num_idxs
                ).then_inc(gather_sems[gc % NSEMS], 1)

        @block.tensor
        def _(tensor):
            if not do_pe:
                return
            tensor.wait_ge(load_sem, 16)
            for gk in range(repeat * NCHUNK * TPC):
                gc, t = divmod(gk, TPC)
                q = gk // 4          # global 4-tile copy block
                if t == 0 and do_gather:
                    tensor.wait_ge(gather_sems[gc % NSEMS], rnd(gc))
                if gk % 4 == 0 and q >= NPSUM and do_copy:
                    tensor.wait_ge(copydone_sems[(q - NPSUM) % NSEMS],
                                   rnd(q - NPSUM))
                mm = tensor.matmul(
                    psum[q % NPSUM][:, (gk % 4) * 128:(gk % 4) * 128 + 128],
                    gbuf[:, gc % GSLOTS, t * 128:(t + 1) * 128],
                    ident_sb[:],
                    is_transpose=True,
                    start=True,
                    stop=True,
                )
                mm.then_inc(blkready_sems[q % NSEMS], 1)

        def copy_body(eng, parity):
            if not do_copy:
                return
            for q in range(repeat * BLOCKS):
                if q % 2 != parity:
                    continue
                gq = q // BPG        # global store group
                if do_pe:
                    eng.wait_ge(blkready_sems[q % NSEMS], 4 * rnd(q))
                if gq >= NBUFS and do_store:
                    eng.wait_ge(store_sems[(gq - NBUFS) % NSEMS],
                                16 * rnd(gq - NBUFS))
                qq = q % BPG
                dst = buf[:, gq % NBUFS, qq * 4:(qq + 1) * 4]
                src = psum[q % NPSUM][:]
                cp = (eng.copy(dst, src) if parity == 0
                      else eng.tensor_copy(dst, src))
                cp.then_inc(copydone_sems[q % NSEMS], 1)

        @block.scalar
        def _(scalar):
            copy_body(scalar, 0)

        @block.vector
        def _(vector):
            copy_body(vector, 1)

    nc.compile()
    return nc


def make_in_maps(nodes, children):
    """Host-side shard + layout preprocessing.

    Gather stream position c = s*8192 + t*128 + p produces output row
    s*8192 + p*64 + t (so the PE tile transpose + contiguous store land
    rows in natural order). ap_gather unwraps indices per 16-partition
    group as idx[16g + k%16, base + k//16], identical for all 8 groups.
    """
    nodes_z = np.ascontiguousarray(np.asarray(nodes), dtype=np.float32).copy()
    nodes_z[:, 0, :] = 0.0
    ch = np.asarray(children).astype(np.int64)
    ident = np.ascontiguousarray(np.eye(128, dtype=np.float32))

    in_maps = []
    for core in range(N_CORES):
        nb = nodes_z[core * B_PER_CORE:(core + 1) * B_PER_CORE]
        # feature-major table [128, 4096]
        table = np.ascontiguousarray(
            nb.transpose(2, 0, 1).reshape(F, TBL_COLS).astype(np.float32)
        )
        cb = ch[core * B_PER_CORE:(core + 1) * B_PER_CORE]
        # batch-local indices: each ap_gather call uses its batch's
        # 2048-column table slice
        flat = cb.reshape(ROWS_PER_CORE)
        # row r = s*8192 + p*64 + t  ->  stream position s*8192 + t*128 + p
        a = flat.reshape(N_GROUPS, 128, GT)          # [s, p, t]
        stream = a.transpose(0, 2, 1).reshape(ROWS_PER_CORE)  # [s, t, p]
        # wrap-16: idx16[l, j] = stream[j*16 + l]
        w = stream.reshape(ROWS_PER_CORE // 16, 16).T        # [16, 8192]
        idx16 = np.tile(w, (8, 1)).astype(np.int16)
        in_maps.append(
            {"table": table, "idxs": np.ascontiguousarray(idx16),
             "ident": ident}
        )
    return in_maps


_NC_CACHE = None


def kernel(nodes, children, feature_size=None):
    global _NC_CACHE
    if _NC_CACHE is None:
        _NC_CACHE = build_nc()
    nc = _NC_CACHE

    in_maps = make_in_maps(nodes, children)
    res = run_bass_kernel_spmd(nc, in_maps, list(range(N_CORES))).results

    out = np.empty((B, N, C, F), np.float32)
    for core in range(N_CORES):
        out[core * B_PER_CORE:(core + 1) * B_PER_CORE] = (
            res[core]["out"].reshape(B_PER_CORE, N, C, F)
        )
    return out


# revision 4
# speedup vs baseline: 45.3546x; 1.0322x over previous
"""Trainium2 Bass kernel v2 for nn_CHILDREN_TENSOR (gnn_message_passing).

Problem: nodes [16, 2048, 128] f32, children [16, 2048, 32] int32.
Output [16, 2048, 32, 128] f32: out[b, n, c, :] = lookup[b, children[b,n,c], :]
where lookup = nodes with row 0 zeroed per batch.

Strategy (data-parallel over batch, 2 per core): keep the whole node
table in SBUF feature-major ([128 feat-partitions x 4096 rows]) and do
the gather ON-CHIP with gpsimd ap_gather (free-dim gather, identical
index stream for all partition groups). Gathered columns are transposed
back to row-major 128x128 tiles on the PE (is_transpose matmul against a
fp32 identity - a pure permutation, bit-exact), drained PSUM->SBUF by
ACT and DVE in 512-column blocks, and stored to HBM as 4 MB
contiguous-per-partition HWDGE writes. DMA then only carries the 2 MB
table + 2 MB indices in and the 64 MB result out - the 64 MB random
HBM gather read of the dma_gather design is gone.
"""

import sys

for _p in ("/opt/trn_rl_repo",):
    if _p not in sys.path:
        sys.path.insert(0, _p)

from contextlib import ExitStack

import numpy as np

import concourse.bacc as bacc
import concourse.mybir as mybir
from concourse.bass_utils import run_bass_kernel_spmd

# Problem constants (hardcoded per harness contract).
B, N, C, F = 16, 2048, 32, 128
N_CORES = 8
B_PER_CORE = B // N_CORES            # 2
ROWS_PER_CORE = B_PER_CORE * N * C   # 131072 output rows per core
TBL_COLS = B_PER_CORE * N            # 4096 table columns (feature-major)

CHUNK = 2048                         # gather columns per ap_gather call
NCHUNK = ROWS_PER_CORE // CHUNK      # 64 per iteration
TPC = CHUNK // 128                   # 16 transpose tiles per chunk
GROUP_ROWS = 8192                    # rows per store
GT = GROUP_ROWS // 128               # 64 tiles per store group
N_GROUPS = ROWS_PER_CORE // GROUP_ROWS               # 16 stores per iteration
BLOCKS = ROWS_PER_CORE // 512        # 256 copy blocks (4 tiles) per iteration
BPG = GT // 4                        # 16 copy blocks per store group

NSEMS = 16
NBUFS = 3                            # store-group SBUF buffers
NPSUM = 6                            # rotating PSUM banks of [128, 512]
GSLOTS = 6                           # gather-chunk pipeline depth


def build_nc(repeat=1, timing_build=False, mode="full",
             do_gather=True, do_pe=True, do_copy=True, do_store=True):
    if mode == "store":
        do_gather = do_pe = do_copy = False
    elif mode == "gather":
        do_pe = do_copy = do_store = False
    elif mode == "nostore":
        do_store = False
    elif mode == "compute":
        do_gather = do_store = False
    nc = bacc.Bacc("TRN2", debug=False, target_bir_lowering=False)

    table = nc.dram_tensor(
        "table", [128, TBL_COLS], mybir.dt.float32,
        kind="Internal" if timing_build else "ExternalInput",
    )
    idxs = nc.dram_tensor(
        "idxs", [128, ROWS_PER_CORE // 16], mybir.dt.int16, kind="ExternalInput"
    )
    ident = nc.dram_tensor("ident", [128, 128], mybir.dt.float32,
                           kind="ExternalInput")
    out = nc.dram_tensor(
        "out", [ROWS_PER_CORE, F], mybir.dt.float32,
        kind="Internal" if timing_build else "ExternalOutput",
    )
    tok = (
        nc.dram_tensor("tok", [1, F], mybir.dt.float32, kind="ExternalOutput")
        if timing_build else None
    )

    with (
        nc.sbuf_tensor("table_sb", [128, TBL_COLS], mybir.dt.float32) as table_sb,
        nc.sbuf_tensor(
            "idx_sb", [128, ROWS_PER_CORE // 16], mybir.dt.int16
        ) as idx_sb,
        nc.sbuf_tensor("ident_sb", [128, 128], mybir.dt.float32) as ident_sb,
        nc.sbuf_tensor("gbuf", [128, GSLOTS, CHUNK], mybir.dt.float32) as gbuf,
        nc.sbuf_tensor("buf", [128, NBUFS, GT, F], mybir.dt.float32) as buf,
        nc.semaphore("load_sem") as load_sem,
        nc.semaphore("qstore_sem") as qstore_sem,
        nc.semaphore("tb0_sem") as tb0_sem,
        nc.semaphore("tb1_sem") as tb1_sem,
        nc.semaphore("idxp0") as idxp0,
        nc.semaphore("idxp1") as idxp1,
        nc.semaphore("idxp2") as idxp2,
        nc.semaphore("idxp3") as idxp3,
        ExitStack() as stack,
        nc.Block() as block,
    ):
        psum = [
            stack.enter_context(
                nc.psum_tensor(f"ps{i}", [128, 512], mybir.dt.float32)
            )
            for i in range(NPSUM)
        ]
        gather_sems = [
            stack.enter_context(nc.semaphore(f"gather_sem{i}"))
            for i in range(NSEMS)
        ]
        blkready_sems = [
            stack.enter_context(nc.semaphore(f"blkready_sem{i}"))
            for i in range(NSEMS)
        ]
        copydone_sems = [
            stack.enter_context(nc.semaphore(f"copydone_sem{i}"))
            for i in range(NSEMS)
        ]
        store_sems = [
            stack.enter_context(nc.semaphore(f"store_sem{i}"))
            for i in range(NSEMS)
        ]

        def rnd(i):
            return i // NSEMS + 1

        IDX_PIECES = 4
        IP_COLS = (ROWS_PER_CORE // 16) // IDX_PIECES
        CHUNKS_PER_PIECE = NCHUNK // IDX_PIECES

        @block.sync
        def _(sync):
            # batch-0 table + ident + idx piece 0 first so the gather and
            # PE pipelines start as early as possible; the rest streams in
            # behind them on dedicated semaphores.
            sync.dma_start(table_sb[:, :N], table[:, :N]).then_inc(tb0_sem, 16)
            sync.dma_start(ident_sb[:], ident[:]).then_inc(load_sem, 16)
            idxp_sems = [idxp0, idxp1, idxp2, idxp3]
            sync.dma_start(idx_sb[:, :IP_COLS],
                           idxs[:, :IP_COLS]).then_inc(idxp0, 16)
            sync.dma_start(table_sb[:, N:], table[:, N:]).then_inc(tb1_sem, 16)
            for ip in range(1, IDX_PIECES):
                sync.dma_start(
                    idx_sb[:, ip * IP_COLS:(ip + 1) * IP_COLS],
                    idxs[:, ip * IP_COLS:(ip + 1) * IP_COLS],
                ).then_inc(idxp_sems[ip], 16)
            # Merged 2-D APs on both sides: per partition one contiguous
            # 32 KB run -> large descriptors.
            out_v = out.rearrange("(s p gf) f -> s p (gf f)", p=128, gf=GT)
            buf_v = buf.rearrange("p n g f -> p n (g f)")
            if do_store:
                # copy-block-granular stores (512 KB): each waits exactly
                # one copydone, so the DMA engines start right behind the
                # first copied block and the drain tail is one block, not
                # a whole 4 MB group. HWDGE is FIFO per issuing engine, so
                # only the last block-store of a group carries the group's
                # buf-recycle semaphore.
                bsz = 4 * F      # one copy block = 4 tiles = 512 elements
                for gs in range(repeat * N_GROUPS):
                    s = gs % N_GROUPS
                    for k in range(BPG):
                        q = gs * BPG + k
                        if do_copy:
                            sync.wait_ge(copydone_sems[q % NSEMS], rnd(q))
                        st = sync.dma_start(
                            out_v[s][:, k * bsz:(k + 1) * bsz],
                            buf_v[:, gs % NBUFS, k * bsz:(k + 1) * bsz],
                        )
                        if k == BPG - 1:
                            st.then_inc(store_sems[gs % NSEMS], 16)
                        else:
                            st.then_inc(qstore_sem, 16)
                for i in range(NSEMS):
                    sync.wait_ge(
                        store_sems[i], 16 * (repeat * N_GROUPS // NSEMS)
                    )
            elif do_copy:
                for i in range(NSEMS):
                    sync.wait_ge(
                        copydone_sems[i], repeat * BLOCKS // NSEMS
                    )
            elif do_gather:
                for i in range(NSEMS):
                    sync.wait_ge(
                        gather_sems[i], repeat * NCHUNK // NSEMS
                    )
            if tok is not None:
                sync.dma_start(tok[:], buf[:1, 0, 0, :]).then_inc(load_sem, 16)
                sync.wait_ge(load_sem, 32)

        @block.gpsimd
        def _(gpsimd):
            if not do_gather:
                return
            idxp_sems = [idxp0, idxp1, idxp2, idxp3]
            gpsimd.wait_ge(tb0_sem, 16)
            for gc in range(repeat * NCHUNK):
                c = gc % NCHUNK
                if gc < NCHUNK and c % CHUNKS_PER_PIECE == 0:
                    gpsimd.wait_ge(idxp_sems[c // CHUNKS_PER_PIECE], 16)
                if gc == NCHUNK // B_PER_CORE:
                    gpsimd.wait_ge(tb1_sem, 16)
                bpc = CHUNK // 512
                if gc >= GSLOTS and do_copy:
                    for q in range((gc - GSLOTS) * bpc, (gc - GSLOTS + 1) * bpc):
                        gpsimd.wait_ge(copydone_sems[q % NSEMS], rnd(q))
                # chunks never straddle the batch boundary; use the
                # per-batch 2048-column table slice with batch-local idxs
                b = c // (NCHUNK // B_PER_CORE)
                gpsimd.ap_gather(
                    gbuf[:, gc % GSLOTS],
                    table_sb[:, b * N:(b + 1) * N],
                    idx_sb[:, c * (CHUNK // 16):(c + 1) * (CHUNK // 16)],
                    128,          # channels
                    N,            # num_elems
                    1,            # d
                    CHUNK,        # num_idxs
                ).then_inc(gather_sems[gc % NSEMS], 1)

        @block.tensor
        def _(tensor):
            if not do_pe:
                return
            tensor.wait_ge(load_sem, 16)
            for gk in range(repeat * NCHUNK * TPC):
                gc, t = divmod(gk, TPC)
                q = gk // 4          # global 4-tile copy block
                if t == 0 and do_gather:
                    tensor.wait_ge(gather_sems[gc % NSEMS], rnd(gc))
                if gk % 4 == 0 and q >= NPSUM and do_copy:
                    tensor.wait_ge(copydone_sems[(q - NPSUM) % NSEMS],
                                   rnd(q - NPSUM))
                mm = tensor.matmul(
                    psum[q % NPSUM][:, (gk % 4) * 128:(gk % 4) * 128 + 128],
                    gbuf[:, gc % GSLOTS, t * 128:(t + 1) * 128],
                    ident_sb[:],
                    is_transpose=True,
                    start=True,
                    stop=True,
                )
                mm.then_inc(blkready_sems[q % NSEMS], 1)

        def copy_body(eng, parity):
            if not do_copy:
                return
            for q in range(repeat * BLOCKS):
                if q % 2 != parity:
                    continue
                gq = q // BPG        # global store group
                if do_pe:
                    eng.wait_ge(blkready_sems[q % NSEMS], 4 * rnd(q))
                if gq >= NBUFS and do_store:
                    eng.wait_ge(store_sems[(gq - NBUFS) % NSEMS],
                                16 * rnd(gq - NBUFS))
                qq = q % BPG
                dst = buf[:, gq % NBUFS, qq * 4:(qq + 1) * 4]
                src = psum[q % NPSUM][:]
                cp = (eng.copy(dst, src) if parity == 0
                      else eng.tensor_copy(dst, src))
                cp.then_inc(copydone_sems[q % NSEMS], 1)

        @block.scalar
        def _(scalar):
            copy_body(scalar, 0)

        @block.vector
        def _(vector):
            copy_body(vector, 1)

    nc.compile()
    return nc


def make_in_maps(nodes, children):
    """Host-side shard + layout preprocessing.

    Gather stream position c = s*8192 + t*128 + p produces output row
    s*8192 + p*64 + t (so the PE tile transpose + contiguous store land
    rows in natural order). ap_gather unwraps indices per 16-partition
    group as idx[16g + k%16, base + k//16], identical for all 8 groups.
    """
    nodes_z = np.ascontiguousarray(np.asarray(nodes), dtype=np.float32).copy()
    nodes_z[:, 0, :] = 0.0
    ch = np.asarray(children).astype(np.int64)
    ident = np.ascontiguousarray(np.eye(128, dtype=np.float32))

    in_maps = []
    for core in range(N_CORES):
        nb = nodes_z[core * B_PER_CORE:(core + 1) * B_PER_CORE]
        # feature-major table [128, 4096]
        table = np.ascontiguousarray(
            nb.transpose(2, 0, 1).reshape(F, TBL_COLS).astype(np.float32)
        )
        cb = ch[core * B_PER_CORE:(core + 1) * B_PER_CORE]
        # batch-local indices: each ap_gather call uses its batch's
        # 2048-column table slice
        flat = cb.reshape(ROWS_PER_CORE)
        # row r = s*8192 + p*64 + t  ->  stream position s*8192 + t*128 + p
        a = flat.reshape(N_GROUPS, 128, GT)          # [s, p, t]
        stream = a.transpose(0, 2, 1).reshape(ROWS_PER_CORE)  # [s, t, p]
        # wrap-16: idx16[l, j] = stream[j*16 + l]
        w = stream.reshape(ROWS_PER_CORE // 16, 16).T        # [16, 8192]
        idx16 = np.tile(w, (8, 1)).astype(np.int16)
        in_maps.append(
            {"table": table, "idxs": np.ascontiguousarray(idx16),
             "ident": ident}
        )
    return in_maps


_NC_CACHE = None


def kernel(nodes, children, feature_size=None):
    global _NC_CACHE
    if _NC_CACHE is None:
        _NC_CACHE = build_nc()
    nc = _NC_CACHE

    in_maps = make_in_maps(nodes, children)
    res = run_bass_kernel_spmd(nc, in_maps, list(range(N_CORES))).results

    out = np.empty((B, N, C, F), np.float32)
    for core in range(N_CORES):
        out[core * B_PER_CORE:(core + 1) * B_PER_CORE] = (
            res[core]["out"].reshape(B_PER_CORE, N, C, F)
        )
    return out


# revision 5
# speedup vs baseline: 46.0988x; 1.0164x over previous
"""Trainium2 Bass kernel v2 for nn_CHILDREN_TENSOR (gnn_message_passing).

Problem: nodes [16, 2048, 128] f32, children [16, 2048, 32] int32.
Output [16, 2048, 32, 128] f32: out[b, n, c, :] = lookup[b, children[b,n,c], :]
where lookup = nodes with row 0 zeroed per batch.

Strategy (data-parallel over batch, 2 per core): keep the whole node
table in SBUF feature-major ([128 feat-partitions x 4096 rows]) and do
the gather ON-CHIP with gpsimd ap_gather (free-dim gather, identical
index stream for all partition groups). Gathered columns are transposed
back to row-major 128x128 tiles on the PE (is_transpose matmul against a
fp32 identity - a pure permutation, bit-exact), drained PSUM->SBUF by
ACT and DVE in 512-column blocks, and stored to HBM as 4 MB
contiguous-per-partition HWDGE writes. DMA then only carries the 2 MB
table + 2 MB indices in and the 64 MB result out - the 64 MB random
HBM gather read of the dma_gather design is gone.
"""

import sys

for _p in ("/opt/trn_rl_repo",):
    if _p not in sys.path:
        sys.path.insert(0, _p)

from contextlib import ExitStack

import numpy as np

import concourse.bacc as bacc
import concourse.mybir as mybir
from concourse.bass_utils import run_bass_kernel_spmd

# Problem constants (hardcoded per harness contract).
B, N, C, F = 16, 2048, 32, 128
N_CORES = 8
B_PER_CORE = B // N_CORES            # 2
ROWS_PER_CORE = B_PER_CORE * N * C   # 131072 output rows per core
TBL_COLS = B_PER_CORE * N            # 4096 table columns (feature-major)

CHUNK = 2048                         # gather columns per ap_gather call
NCHUNK = ROWS_PER_CORE // CHUNK      # 64 per iteration
TPC = CHUNK // 128                   # 16 transpose tiles per chunk
GROUP_ROWS = 8192                    # rows per store
GT = GROUP_ROWS // 128               # 64 tiles per store group
N_GROUPS = ROWS_PER_CORE // GROUP_ROWS               # 16 stores per iteration
BLOCKS = ROWS_PER_CORE // 512        # 256 copy blocks (4 tiles) per iteration
BPG = GT // 4                        # 16 copy blocks per store group

NSEMS = 16
NBUFS = 3                            # store-group SBUF buffers
NPSUM = 6                            # rotating PSUM banks of [128, 512]
GSLOTS = 6                           # gather-chunk pipeline depth


def build_nc(repeat=1, timing_build=False, mode="full",
             do_gather=True, do_pe=True, do_copy=True, do_store=True):
    if mode == "store":
        do_gather = do_pe = do_copy = False
    elif mode == "gather":
        do_pe = do_copy = do_store = False
    elif mode == "nostore":
        do_store = False
    elif mode == "compute":
        do_gather = do_store = False
    nc = bacc.Bacc("TRN2", debug=False, target_bir_lowering=False)

    table = nc.dram_tensor(
        "table", [128, TBL_COLS], mybir.dt.float32,
        kind="Internal" if timing_build else "ExternalInput",
    )
    idxs = nc.dram_tensor(
        "idxs", [128, ROWS_PER_CORE // 16], mybir.dt.int16, kind="ExternalInput"
    )
    ident = nc.dram_tensor("ident", [128, 128], mybir.dt.float32,
                           kind="ExternalInput")
    out = nc.dram_tensor(
        "out", [ROWS_PER_CORE, F], mybir.dt.float32,
        kind="Internal" if timing_build else "ExternalOutput",
    )
    tok = (
        nc.dram_tensor("tok", [1, F], mybir.dt.float32, kind="ExternalOutput")
        if timing_build else None
    )

    with (
        nc.sbuf_tensor("table_sb", [128, TBL_COLS], mybir.dt.float32) as table_sb,
        nc.sbuf_tensor(
            "idx_sb", [128, ROWS_PER_CORE // 16], mybir.dt.int16
        ) as idx_sb,
        nc.sbuf_tensor("ident_sb", [128, 128], mybir.dt.float32) as ident_sb,
        nc.sbuf_tensor("gbuf", [128, GSLOTS, CHUNK], mybir.dt.float32) as gbuf,
        nc.sbuf_tensor("buf", [128, NBUFS, GT, F], mybir.dt.float32) as buf,
        nc.semaphore("load_sem") as load_sem,
        nc.semaphore("qstore_sem") as qstore_sem,
        nc.semaphore("tb0_sem") as tb0_sem,
        nc.semaphore("tb1_sem") as tb1_sem,
        nc.semaphore("idxp0") as idxp0,
        nc.semaphore("idxp1") as idxp1,
        nc.semaphore("idxp2") as idxp2,
        nc.semaphore("idxp3") as idxp3,
        ExitStack() as stack,
        nc.Block() as block,
    ):
        psum = [
            stack.enter_context(
                nc.psum_tensor(f"ps{i}", [128, 512], mybir.dt.float32)
            )
            for i in range(NPSUM)
        ]
        scratch_ps = stack.enter_context(
            nc.psum_tensor("ps_warm", [128, 128], mybir.dt.float32)
        )
        gather_sems = [
            stack.enter_context(nc.semaphore(f"gather_sem{i}"))
            for i in range(NSEMS)
        ]
        blkready_sems = [
            stack.enter_context(nc.semaphore(f"blkready_sem{i}"))
            for i in range(NSEMS)
        ]
        copydone_sems = [
            stack.enter_context(nc.semaphore(f"copydone_sem{i}"))
            for i in range(NSEMS)
        ]
        store_sems = [
            stack.enter_context(nc.semaphore(f"store_sem{i}"))
            for i in range(NSEMS)
        ]

        def rnd(i):
            return i // NSEMS + 1

        IDX_PIECES = 4
        IP_COLS = (ROWS_PER_CORE // 16) // IDX_PIECES
        CHUNKS_PER_PIECE = NCHUNK // IDX_PIECES

        @block.sync
        def _(sync):
            # batch-0 table + ident + idx piece 0 first so the gather and
            # PE pipelines start as early as possible; the rest streams in
            # behind them on dedicated semaphores.
            sync.dma_start(table_sb[:, :N], table[:, :N]).then_inc(tb0_sem, 16)
            sync.dma_start(ident_sb[:], ident[:]).then_inc(load_sem, 16)
            idxp_sems = [idxp0, idxp1, idxp2, idxp3]
            sync.dma_start(idx_sb[:, :IP_COLS],
                           idxs[:, :IP_COLS]).then_inc(idxp0, 16)
            sync.dma_start(table_sb[:, N:], table[:, N:]).then_inc(tb1_sem, 16)
            for ip in range(1, IDX_PIECES):
                sync.dma_start(
                    idx_sb[:, ip * IP_COLS:(ip + 1) * IP_COLS],
                    idxs[:, ip * IP_COLS:(ip + 1) * IP_COLS],
                ).then_inc(idxp_sems[ip], 16)
            # Merged 2-D APs on both sides: per partition one contiguous
            # 32 KB run -> large descriptors.
            out_v = out.rearrange("(s p gf) f -> s p (gf f)", p=128, gf=GT)
            buf_v = buf.rearrange("p n g f -> p n (g f)")
            if do_store:
                # copy-block-granular stores (512 KB): each waits exactly
                # one copydone, so the DMA engines start right behind the
                # first copied block and the drain tail is one block, not
                # a whole 4 MB group. HWDGE is FIFO per issuing engine, so
                # only the last block-store of a group carries the group's
                # buf-recycle semaphore.
                SB = 1
                bsz = SB * 4 * F     # store unit = SB copy blocks
                for gs in range(repeat * N_GROUPS):
                    s = gs % N_GROUPS
                    for k in range(BPG // SB):
                        for q in range(gs * BPG + k * SB,
                                       gs * BPG + (k + 1) * SB):
                            if do_copy:
                                sync.wait_ge(copydone_sems[q % NSEMS], rnd(q))
                        st = sync.dma_start(
                            out_v[s][:, k * bsz:(k + 1) * bsz],
                            buf_v[:, gs % NBUFS, k * bsz:(k + 1) * bsz],
                        )
                        if k == BPG // SB - 1:
                            st.then_inc(store_sems[gs % NSEMS], 16)
                        else:
                            st.then_inc(qstore_sem, 16)
                for i in range(NSEMS):
                    sync.wait_ge(
                        store_sems[i], 16 * (repeat * N_GROUPS // NSEMS)
                    )
            elif do_copy:
                for i in range(NSEMS):
                    sync.wait_ge(
                        copydone_sems[i], repeat * BLOCKS // NSEMS
                    )
            elif do_gather:
                for i in range(NSEMS):
                    sync.wait_ge(
                        gather_sems[i], repeat * NCHUNK // NSEMS
                    )
            if tok is not None:
                sync.dma_start(tok[:], buf[:1, 0, 0, :]).then_inc(load_sem, 16)
                sync.wait_ge(load_sem, 32)

        @block.gpsimd
        def _(gpsimd):
            if not do_gather:
                return
            idxp_sems = [idxp0, idxp1, idxp2, idxp3]
            gpsimd.wait_ge(tb0_sem, 16)
            for gc in range(repeat * NCHUNK):
                c = gc % NCHUNK
                if gc < NCHUNK and c % CHUNKS_PER_PIECE == 0:
                    gpsimd.wait_ge(idxp_sems[c // CHUNKS_PER_PIECE], 16)
                if gc == NCHUNK // B_PER_CORE:
                    gpsimd.wait_ge(tb1_sem, 16)
                bpc = CHUNK // 512
                if gc >= GSLOTS and do_copy:
                    for q in range((gc - GSLOTS) * bpc, (gc - GSLOTS + 1) * bpc):
                        gpsimd.wait_ge(copydone_sems[q % NSEMS], rnd(q))
                # chunks never straddle the batch boundary; use the
                # per-batch 2048-column table slice with batch-local idxs
                b = c // (NCHUNK // B_PER_CORE)
                gpsimd.ap_gather(
                    gbuf[:, gc % GSLOTS],
                    table_sb[:, b * N:(b + 1) * N],
                    idx_sb[:, c * (CHUNK // 16):(c + 1) * (CHUNK // 16)],
                    128,          # channels
                    N,            # num_elems
                    1,            # d
                    CHUNK,        # num_idxs
                ).then_inc(gather_sems[gc % NSEMS], 1)

        @block.tensor
        def _(tensor):
            if not do_pe:
                return
            tensor.wait_ge(load_sem, 16)
            # p-state warmup: keep the PE busy on throwaway identity
            # transposes while the first gather chunk is in flight, so
            # real tiles run at full clock from the start.
            NWARM = 12
            if do_gather:
                for _ in range(NWARM):
                    tensor.matmul(
                        scratch_ps[:], ident_sb[:], ident_sb[:],
                        is_transpose=True, start=True, stop=True,
                    )
            for gk in range(repeat * NCHUNK * TPC):
                gc, t = divmod(gk, TPC)
                q = gk // 4          # global 4-tile copy block
                if t == 0 and do_gather:
                    tensor.wait_ge(gather_sems[gc % NSEMS], rnd(gc))
                if gk % 4 == 0 and q >= NPSUM and do_copy:
                    tensor.wait_ge(copydone_sems[(q - NPSUM) % NSEMS],
                                   rnd(q - NPSUM))
                mm = tensor.matmul(
                    psum[q % NPSUM][:, (gk % 4) * 128:(gk % 4) * 128 + 128],
                    gbuf[:, gc % GSLOTS, t * 128:(t + 1) * 128],
                    ident_sb[:],
                    is_transpose=True,
                    start=True,
                    stop=True,
                )
                mm.then_inc(blkready_sems[q % NSEMS], 1)

        def copy_body(eng, parity):
            if not do_copy:
                return
            for q in range(repeat * BLOCKS):
                if q % 2 != parity:
                    continue
                gq = q // BPG        # global store group
                if do_pe:
                    eng.wait_ge(blkready_sems[q % NSEMS], 4 * rnd(q))
                if gq >= NBUFS and do_store:
                    eng.wait_ge(store_sems[(gq - NBUFS) % NSEMS],
                                16 * rnd(gq - NBUFS))
                qq = q % BPG
                dst = buf[:, gq % NBUFS, qq * 4:(qq + 1) * 4]
                src = psum[q % NPSUM][:]
                cp = (eng.copy(dst, src) if parity == 0
                      else eng.tensor_copy(dst, src))
                cp.then_inc(copydone_sems[q % NSEMS], 1)

        @block.scalar
        def _(scalar):
            copy_body(scalar, 0)

        @block.vector
        def _(vector):
            copy_body(vector, 1)

    nc.compile()
    return nc


def make_in_maps(nodes, children):
    """Host-side shard + layout preprocessing.

    Gather stream position c = s*8192 + t*128 + p produces output row
    s*8192 + p*64 + t (so the PE tile transpose + contiguous store land
    rows in natural order). ap_gather unwraps indices per 16-partition
    group as idx[16g + k%16, base + k//16], identical for all 8 groups.
    """
    nodes_z = np.ascontiguousarray(np.asarray(nodes), dtype=np.float32).copy()
    nodes_z[:, 0, :] = 0.0
    ch = np.asarray(children).astype(np.int64)
    ident = np.ascontiguousarray(np.eye(128, dtype=np.float32))

    in_maps = []
    for core in range(N_CORES):
        nb = nodes_z[core * B_PER_CORE:(core + 1) * B_PER_CORE]
        # feature-major table [128, 4096]
        table = np.ascontiguousarray(
            nb.transpose(2, 0, 1).reshape(F, TBL_COLS).astype(np.float32)
        )
        cb = ch[core * B_PER_CORE:(core + 1) * B_PER_CORE]
        # batch-local indices: each ap_gather call uses its batch's
        # 2048-column table slice
        flat = cb.reshape(ROWS_PER_CORE)
        # row r = s*8192 + p*64 + t  ->  stream position s*8192 + t*128 + p
        a = flat.reshape(N_GROUPS, 128, GT)          # [s, p, t]
        stream = a.transpose(0, 2, 1).reshape(ROWS_PER_CORE)  # [s, t, p]
        # wrap-16: idx16[l, j] = stream[j*16 + l]
        w = stream.reshape(ROWS_PER_CORE // 16, 16).T        # [16, 8192]
        idx16 = np.tile(w, (8, 1)).astype(np.int16)
        in_maps.append(
            {"table": table, "idxs": np.ascontiguousarray(idx16),
             "ident": ident}
        )
    return in_maps


_NC_CACHE = None


def kernel(nodes, children, feature_size=None):
    global _NC_CACHE
    if _NC_CACHE is None:
        _NC_CACHE = build_nc()
    nc = _NC_CACHE

    in_maps = make_in_maps(nodes, children)
    res = run_bass_kernel_spmd(nc, in_maps, list(range(N_CORES))).results

    out = np.empty((B, N, C, F), np.float32)
    for core in range(N_CORES):
        out[core * B_PER_CORE:(core + 1) * B_PER_CORE] = (
            res[core]["out"].reshape(B_PER_CORE, N, C, F)
        )
    return out
